# revision 1
# baseline (speedup 1.0000x reference)
"""HDC Level Encoder kernel for 8 Trainium2 NeuronCores.

Strategy (D=100000 hypervector dim sharded 8 ways, padded to 12800/core):
  - level-table lookups as one-hot matmuls on PE (tables stream once, bf16-exact
    for +-1 entries); x/y/z lookups accumulate the bundle sum directly in PSUM.
  - bind with time hv on DVE, multibind product over the N=128 window via PE
    transpose to d-on-partitions layout + pairwise DVE fold tree.
  - Sinusoid einsum as fp32 PE matmuls against a host-pretransposed [600, D]
    weight stack with a block-diagonal feature matrix.
  - cos(p+b)*sin(p) via ScalarE Sin with explicit range reduction in cycle
    units: m = mod(u,1); Sin(2*pi*m - pi) = -sin(2*pi*u); the two minus signs
    cancel in the product.
  - combine + hard_quantize on DVE, transpose back, DMA out.

Host does only O(N*levels) prep: index math (bit-identical to the reference's
f32 ops), one-hot construction, weight restacking/padding, and sharding.
"""

import sys

for _p in ("/opt/trn_rl_repo",):
    if _p not in sys.path:
        sys.path.insert(0, _p)

import numpy as np

import concourse.bacc as bacc
import concourse.mybir as mybir
import concourse.tile as tile
from concourse import bass_utils, masks

BF16 = mybir.dt.np(mybir.dt.bfloat16)

D = 100000          # true hypervector dim
NCORES = 8
DC = 12800          # per-core padded dim
DP = DC * NCORES    # 102400
N = 128             # window length
LEVELS = 100
TIMESTAMPS = 128
CH = 25             # chunks of 512 per core
CW = 512            # chunk width
NSUB = CH * 4       # 100 sub-chunks of 128
NK = 24             # sinusoid kernels (6 big + 18 small)
KROWS = 600         # stacked contraction dim (6*91 + 18*3)

F32 = mybir.dt.float32
I32 = mybir.dt.int32
BF = mybir.dt.bfloat16
AF = mybir.AluOpType

_TWO_PI = np.float32(2.0 * np.pi)
_PI = np.float32(np.pi)
_INV_2PI = np.float32(1.0 / (2.0 * np.pi))

_nc_cache = {}


def _build_nc():
    nc = bacc.Bacc("TRN2", target_bir_lowering=False, debug=False)

    lvlx = nc.dram_tensor("lvlx", [LEVELS, DC], BF, kind="ExternalInput")
    lvly = nc.dram_tensor("lvly", [LEVELS, DC], BF, kind="ExternalInput")
    lvlz = nc.dram_tensor("lvlz", [LEVELS, DC], BF, kind="ExternalInput")
    lvlt = nc.dram_tensor("lvlt", [TIMESTAMPS, DC], BF, kind="ExternalInput")
    ohx = nc.dram_tensor("ohx", [LEVELS, N], BF, kind="ExternalInput")
    ohy = nc.dram_tensor("ohy", [LEVELS, N], BF, kind="ExternalInput")
    ohz = nc.dram_tensor("ohz", [LEVELS, N], BF, kind="ExternalInput")
    oht = nc.dram_tensor("oht", [TIMESTAMPS, N], BF, kind="ExternalInput")
    wstk = nc.dram_tensor("wstk", [KROWS, DC], F32, kind="ExternalInput")
    fbd = nc.dram_tensor("fbd", [KROWS, NK], F32, kind="ExternalInput")
    bsh = nc.dram_tensor("bsh", [N, NSUB * NK], F32, kind="ExternalInput")
    out = nc.dram_tensor("out", [NSUB, N], F32, kind="ExternalOutput")

    kchunks = [(0, 128), (128, 128), (256, 128), (384, 128), (512, 88)]

    with tile.TileContext(nc) as tc:
        with (
            tc.tile_pool(name="const", bufs=1) as constp,
            tc.tile_pool(name="grand", bufs=1) as grandp,
        ):
            ident_bf = constp.tile([128, 128], BF)
            ident_f32 = constp.tile([128, 128], F32)
            masks.make_identity(nc, ident_bf[:])
            masks.make_identity(nc, ident_f32[:])

            ohx_sb = constp.tile([LEVELS, N], BF, tag="ohx")
            ohy_sb = constp.tile([LEVELS, N], BF, tag="ohy")
            ohz_sb = constp.tile([LEVELS, N], BF, tag="ohz")
            oht_sb = constp.tile([TIMESTAMPS, N], BF, tag="oht")
            nc.sync.dma_start(ohx_sb[:], ohx.ap())
            nc.sync.dma_start(ohy_sb[:], ohy.ap())
            nc.sync.dma_start(ohz_sb[:], ohz.ap())
            nc.sync.dma_start(oht_sb[:], oht.ap())

            fbd_sb = []
            for i, (r0, rn) in enumerate(kchunks):
                t = constp.tile([rn, NK], F32, tag=f"fbd{i}")
                nc.sync.dma_start(t[:], fbd.ap()[r0 : r0 + rn, :])
                fbd_sb.append(t)

            bsh_sb = constp.tile([N, NSUB * NK], F32, tag="bsh")
            nc.sync.dma_start(bsh_sb[:], bsh.ap())

            hvg = grandp.tile([128, NSUB], F32, tag="hvg")
            fg = grandp.tile([128, NSUB * NK], F32, tag="fg")

            # ---------------- phase A: lookups, bind, window product ----------
            with (
                tc.tile_pool(name="tabs", bufs=3) as tabp,
                tc.tile_pool(name="binds", bufs=3) as bindp,
                tc.tile_pool(name="folds", bufs=2) as foldp,
                tc.tile_pool(name="psA", bufs=2, space="PSUM") as psa,
            ):
                for c in range(CH):
                    cs = slice(c * CW, (c + 1) * CW)
                    tx = tabp.tile([LEVELS, CW], BF, tag="tx")
                    ty = tabp.tile([LEVELS, CW], BF, tag="ty")
                    tz = tabp.tile([LEVELS, CW], BF, tag="tz")
                    tt = tabp.tile([TIMESTAMPS, CW], BF, tag="tt")
                    nc.sync.dma_start(tx[:], lvlx.ap()[:, cs])
                    nc.sync.dma_start(ty[:], lvly.ap()[:, cs])
                    nc.sync.dma_start(tz[:], lvlz.ap()[:, cs])
                    nc.sync.dma_start(tt[:], lvlt.ap()[:, cs])

                    ps3 = psa.tile([128, CW], F32, tag="ps3")
                    nc.tensor.matmul(ps3[:], ohx_sb[:], tx[:], start=True, stop=False)
                    nc.tensor.matmul(ps3[:], ohy_sb[:], ty[:], start=False, stop=False)
                    nc.tensor.matmul(ps3[:], ohz_sb[:], tz[:], start=False, stop=True)

                    ptl = psa.tile([128, CW], F32, tag="ptl")
                    nc.tensor.matmul(ptl[:], oht_sb[:], tt[:], start=True, stop=True)

                    tl_sb = bindp.tile([128, CW], BF, tag="tl_sb")
                    nc.scalar.copy(tl_sb[:], ptl[:])
                    bind_sb = bindp.tile([128, CW], BF, tag="bind_sb")
                    nc.vector.tensor_mul(bind_sb[:], ps3[:], tl_sb[:])

                    pbt = psa.tile([128, CW], BF, tag="pbt")
                    for s in range(4):
                        ss = slice(s * 128, (s + 1) * 128)
                        nc.tensor.matmul(
                            pbt[:, ss], bind_sb[:, ss], ident_bf[:], is_transpose=True
                        )
                    bt_sb = bindp.tile([128, CW], BF, tag="bt_sb")
                    nc.scalar.copy(bt_sb[:], pbt[:])

                    # pairwise fold over the window dim (free axis, 4 blocks)
                    src = bt_sb[:].rearrange("p (s n) -> p s n", s=4)
                    w = 64
                    while w >= 1:
                        if w == 1:
                            dst_ap = hvg[:, c * 4 : c * 4 + 4].rearrange(
                                "p (s n) -> p s n", n=1
                            )
                        else:
                            t_new = foldp.tile([128, 4 * w], F32, tag=f"fold{w}")
                            dst_ap = t_new[:].rearrange("p (s n) -> p s n", s=4)
                        nc.vector.tensor_mul(
                            dst_ap, src[:, :, 0:w], src[:, :, w : 2 * w]
                        )
                        if w > 1:
                            src = dst_ap
                        w //= 2

            # ---------------- phase B: sinusoid features ----------------------
            with (
                tc.tile_pool(name="wts", bufs=3) as wp,
                tc.tile_pool(name="trig", bufs=3) as trp,
                tc.tile_pool(name="psB", bufs=2, space="PSUM") as psb,
            ):
                for c in range(CH):
                    cs = slice(c * CW, (c + 1) * CW)
                    wts = []
                    for i, (r0, rn) in enumerate(kchunks):
                        t = wp.tile([rn, CW], F32, tag=f"w{i}")
                        nc.sync.dma_start(t[:], wstk.ap()[r0 : r0 + rn, cs])
                        wts.append(t)

                    ppb = psb.tile([NK, CW], F32, tag="ppb")
                    for i in range(5):
                        nc.tensor.matmul(
                            ppb[:],
                            fbd_sb[i][:],
                            wts[i][:],
                            start=(i == 0),
                            stop=(i == 4),
                        )
                    pbk_sb = trp.tile([NK, CW], F32, tag="pbk_sb")
                    nc.scalar.copy(pbk_sb[:], ppb[:])

                    ppt = psb.tile([128, 4 * NK], F32, tag="ppt")
                    for s in range(4):
                        nc.tensor.matmul(
                            ppt[:, s * NK : (s + 1) * NK],
                            pbk_sb[:, s * 128 : (s + 1) * 128],
                            ident_f32[:NK, :NK],
                            is_transpose=True,
                        )

                    fs = slice(c * 4 * NK, (c + 1) * 4 * NK)
                    # range reduction in cycle units: r = u - rint(u) in
                    # [-0.5, 0.5] (DVE f32->int32 copy rounds half-to-even,
                    # and the subtraction is exact), then Sin(2*pi*r) =
                    # sin(2*pi*u) on ScalarE's [-pi, pi] domain.
                    u = trp.tile([128, 4 * NK], F32, tag="u")
                    nc.vector.tensor_scalar_mul(u[:], ppt[:], float(_INV_2PI))
                    i1 = trp.tile([128, 4 * NK], I32, tag="i1")
                    nc.vector.tensor_copy(i1[:], u[:])
                    m1 = trp.tile([128, 4 * NK], F32, tag="m1")
                    nc.vector.tensor_sub(m1[:], u[:], i1[:])
                    nc.vector.tensor_scalar_mul(m1[:], m1[:], float(_TWO_PI))
                    s1 = trp.tile([128, 4 * NK], F32, tag="s1")
                    nc.scalar.activation(
                        s1[:], m1[:], mybir.ActivationFunctionType.Sin
                    )
                    u2 = trp.tile([128, 4 * NK], F32, tag="u2")
                    nc.vector.tensor_add(u2[:], u[:], bsh_sb[:, fs])
                    i2 = trp.tile([128, 4 * NK], I32, tag="i2")
                    nc.vector.tensor_copy(i2[:], u2[:])
                    m2 = trp.tile([128, 4 * NK], F32, tag="m2")
                    nc.vector.tensor_sub(m2[:], u2[:], i2[:])
                    nc.vector.tensor_scalar_mul(m2[:], m2[:], float(_TWO_PI))
                    s2 = trp.tile([128, 4 * NK], F32, tag="s2")
                    nc.scalar.activation(
                        s2[:], m2[:], mybir.ActivationFunctionType.Sin
                    )
                    nc.vector.tensor_mul(fg[:, fs], s2[:], s1[:])

            # ---------------- combine + hard quantize -------------------------
            with (
                tc.tile_pool(name="comb", bufs=1) as cp,
                tc.tile_pool(name="psC", bufs=1, space="PSUM") as psc,
            ):
                f3 = fg[:].rearrange("p (s k) -> p s k", k=NK)

                def f(k):
                    return f3[:, :, k : k + 1]

                def tmp(tag):
                    return cp.tile([128, NSUB], F32, tag=tag, name=tag)

                a1 = tmp("a1")
                a1v = a1[:].rearrange("p (s k) -> p s k", k=1)
                nc.vector.tensor_add(a1v, f(6), f(21))
                nc.vector.tensor_add(a1v, a1v, f(23))
                q1 = tmp("q1")
                q1v = q1[:].rearrange("p (s k) -> p s k", k=1)
                hvv = hvg[:].rearrange("p (s k) -> p s k", k=1)
                nc.vector.tensor_mul(q1v, hvv, a1v)
                a2 = tmp("a2")
                a2v = a2[:].rearrange("p (s k) -> p s k", k=1)
                nc.vector.tensor_add(a2v, f(9), f(10))
                nc.vector.tensor_mul(q1v, q1v, a2v)
                for k in (11, 12, 17, 18):
                    nc.vector.tensor_mul(q1v, q1v, f(k))

                a3 = tmp("a3")
                a3v = a3[:].rearrange("p (s k) -> p s k", k=1)
                nc.vector.tensor_add(a3v, f(6), f(10))
                nc.vector.tensor_add(a3v, a3v, f(11))
                nc.vector.tensor_add(a3v, a3v, f(12))
                p2 = tmp("p2")
                p2v = p2[:].rearrange("p (s k) -> p s k", k=1)
                nc.vector.tensor_mul(p2v, f(0), f(1))
                for k in (2, 3, 4, 5):
                    nc.vector.tensor_mul(p2v, p2v, f(k))
                q2 = tmp("q2")
                q2v = q2[:].rearrange("p (s k) -> p s k", k=1)
                nc.vector.tensor_mul(q2v, hvv, a3v)
                nc.vector.tensor_mul(q2v, q2v, p2v)

                comb = tmp("comb")
                nc.vector.tensor_add(comb[:], q1[:], q2[:])
                outq = tmp("outq")
                nc.vector.tensor_scalar(outq[:], comb[:], 0.0, 2.0, AF.is_gt, AF.mult)
                nc.vector.tensor_scalar(outq[:], outq[:], -1.0, None, AF.add)

                pso = psc.tile([NSUB, 128], F32, tag="pso")
                nc.tensor.matmul(
                    pso[:], outq[:], ident_f32[:], is_transpose=True
                )
                out_sb = cp.tile([NSUB, 128], F32, tag="out_sb")
                nc.scalar.copy(out_sb[:], pso[:])
                nc.sync.dma_start(out.ap(), out_sb[:])

    nc.compile()
    return nc


def _get_nc():
    if "nc" not in _nc_cache:
        _nc_cache["nc"] = _build_nc()
    return _nc_cache["nc"]


def _value_to_index(x, low, high, num):
    """Bit-identical (f32 elementwise IEEE ops) to the reference's jnp math."""
    x = x.astype(np.float32)
    xc = np.clip(x, np.float32(low), np.float32(high))
    t = (xc - np.float32(low)) / np.float32(high - low) * np.float32(num - 1)
    idx = np.round(t)  # round-half-even, same as jnp.round
    return np.clip(idx, 0, num - 1).astype(np.int32)


def _onehot(idx, levels):
    o = np.zeros((levels, N), dtype=BF16)
    o[idx, np.arange(N)] = 1
    return o


def prepare_in_maps(
    input,
    feat,
    level_x,
    level_y,
    level_z,
    level_t,
    W_big,
    b_big,
    W_small,
    b_small,
):
    ix = _value_to_index(input[:, 1], -5.0, 5.0, LEVELS)
    iy = _value_to_index(input[:, 2], -5.0, 5.0, LEVELS)
    iz = _value_to_index(input[:, 3], -5.0, 5.0, LEVELS)
    it = _value_to_index(input[:, 0], 0.0, float(TIMESTAMPS), TIMESTAMPS)
    ohx = _onehot(ix, LEVELS)
    ohy = _onehot(iy, LEVELS)
    ohz = _onehot(iz, LEVELS)
    oht = _onehot(it, TIMESTAMPS)

    featb = feat[:546].reshape(6, 91).astype(np.float32)
    feats = feat[546:600].reshape(18, 3).astype(np.float32)
    fbd = np.zeros((KROWS, NK), dtype=np.float32)
    for k in range(6):
        fbd[k * 91 : (k + 1) * 91, k] = featb[k]
    for k in range(18):
        fbd[546 + k * 3 : 546 + (k + 1) * 3, 6 + k] = feats[k]

    def padD(a):
        w = [(0, 0)] * a.ndim
        w[-1] = (0, DP - D)
        return np.pad(a, w)

    # tables -> bf16 (exact for +-1), padded
    lx = padD(level_x).astype(BF16)
    ly = padD(level_y).astype(BF16)
    lz = padD(level_z).astype(BF16)
    lt = padD(level_t).astype(BF16)

    # W stack [600, DP] f32: rows = (kernel-major, in-feature) of W_big/W_small
    wb = np.ascontiguousarray(W_big.transpose(0, 2, 1)).reshape(546, D)
    ws = np.ascontiguousarray(W_small.transpose(0, 2, 1)).reshape(54, D)
    wstk = padD(np.concatenate([wb, ws], axis=0)).astype(np.float32)

    # b shift in cycles (+0.25 for the cos->sin shift), d-on-partitions layout
    ball = np.concatenate([b_big, b_small], axis=0).astype(np.float64)
    bsh_full = padD((ball / (2.0 * np.pi) + 0.25).astype(np.float32))  # [24, DP]

    in_maps = []
    for ci in range(NCORES):
        ds = slice(ci * DC, (ci + 1) * DC)
        bs = (
            bsh_full[:, ds]
            .reshape(NK, NSUB, 128)
            .transpose(2, 1, 0)
            .reshape(128, NSUB * NK)
        )
        in_maps.append(
            {
                "lvlx": np.ascontiguousarray(lx[:, ds]),
                "lvly": np.ascontiguousarray(ly[:, ds]),
                "lvlz": np.ascontiguousarray(lz[:, ds]),
                "lvlt": np.ascontiguousarray(lt[:, ds]),
                "ohx": ohx,
                "ohy": ohy,
                "ohz": ohz,
                "oht": oht,
                "wstk": np.ascontiguousarray(wstk[:, ds]),
                "fbd": fbd,
                "bsh": np.ascontiguousarray(bs),
            }
        )
    return in_maps


def kernel(**inputs):
    nc = _get_nc()
    in_maps = prepare_in_maps(**inputs)
    _nc_cache["last_in_maps"] = in_maps
    res = bass_utils.run_bass_kernel_spmd(nc, in_maps, core_ids=list(range(NCORES)))
    shards = [res.results[ci]["out"].reshape(-1) for ci in range(NCORES)]
    return np.concatenate(shards)[:D].astype(np.float32)



# revision 2
# speedup vs baseline: 5.2630x; 5.2630x over previous
"""HDC Level Encoder kernel for 8 Trainium2 NeuronCores.

Strategy (D=100000 hypervector dim sharded 8 ways, padded to 12800/core):
  - level-table lookups as one-hot matmuls on PE. Tables and one-hots ship as
    fp8e4 (+-1 and 0/1 are exact); x/y/z lookups accumulate the bundle sum
    directly in PSUM (f32, exact).
  - bind with time hv on DVE, multibind product over the N=128 window via PE
    transpose to d-on-partitions layout + pairwise DVE fold tree (f32, same
    rounding as the reference's f32 product chain).
  - Sinusoid einsum as f32 PE matmuls with the WEIGHT CHUNK STATIONARY
    (lhsT = W[rows, 128 d-cols]) and the block-diagonal feature matrix moving
    (rhs [rows, 24]): the f32 4-cycles/row penalty applies to the 24-wide
    moving operand instead of a 512-wide one, and the result lands directly
    in d-on-partitions layout (no transpose). Accumulation order over the
    contraction rows/chunks is identical to the reference einsum.
  - cos(p+b)*sin(p) via ScalarE Sin with explicit range reduction in cycle
    units: m = u - rint(u); Sin(2*pi*m) = sin(2*pi*u); bsh carries b/(2*pi)
    + 0.25 so the cos becomes the same shifted sin.
  - combine + hard_quantize on DVE, transpose back, DMA out.

All phases run in one chunk loop so table DMA, W DMA, PE, DVE, and ScalarE
work overlap. Host does only O(N*levels + K*D) layout prep: index math
(bit-identical to the reference's f32 ops), one-hot construction, weight
restack/padding, and sharding.
"""

import sys

for _p in ("/opt/trn_rl_repo",):
    if _p not in sys.path:
        sys.path.insert(0, _p)

import numpy as np

import concourse.bacc as bacc
import concourse.mybir as mybir
import concourse.tile as tile
from concourse import bass_utils, masks

F32 = mybir.dt.float32
I32 = mybir.dt.int32
BF = mybir.dt.bfloat16
FP8 = mybir.dt.float8e4
BF16 = mybir.dt.np(BF)
FP8NP = mybir.dt.np(FP8)
AF = mybir.AluOpType

D = 100000          # true hypervector dim
NCORES = 8
DC = 12800          # per-core padded dim
DP = DC * NCORES    # 102400
N = 128             # window length
LEVELS = 100
TIMESTAMPS = 128
TROWS = 3 * LEVELS + TIMESTAMPS  # 428 stacked table rows
CH = 25             # chunks of 512 per core
CW = 512            # chunk width
NSUB = CH * 4       # 100 sub-chunks of 128
NK = 24             # sinusoid kernels (6 big + 18 small)
KROWS = 600         # stacked contraction dim (6*91 + 18*3)

_TWO_PI = np.float32(2.0 * np.pi)
_INV_2PI = np.float32(1.0 / (2.0 * np.pi))

_nc_cache = {}

# (row0, nrows) blocks of the stacked table tensor: x, y, z, t
TBLOCKS = [(0, LEVELS), (LEVELS, LEVELS), (2 * LEVELS, LEVELS), (3 * LEVELS, TIMESTAMPS)]
KCHUNKS = [(0, 128), (128, 128), (256, 128), (384, 128), (512, 88)]


def _build_nc():
    nc = bacc.Bacc("TRN2", target_bir_lowering=False, debug=False)

    tabs = nc.dram_tensor("tabs", [TROWS, DC], FP8, kind="ExternalInput")
    ohs = nc.dram_tensor("ohs", [TROWS, N], FP8, kind="ExternalInput")
    wstk = nc.dram_tensor("wstk", [KROWS, DC], F32, kind="ExternalInput")
    fbd = nc.dram_tensor("fbd", [KROWS, NK], F32, kind="ExternalInput")
    bsh = nc.dram_tensor("bsh", [N, NSUB * NK], F32, kind="ExternalInput")
    out = nc.dram_tensor("out", [NSUB, N], F32, kind="ExternalOutput")

    with tile.TileContext(nc) as tc:
        with (
            tc.tile_pool(name="const", bufs=1) as constp,
            tc.tile_pool(name="grand", bufs=1) as grandp,
        ):
            ident_bf = constp.tile([128, 128], BF)
            masks.make_identity(nc, ident_bf[:])
            ident_f32 = constp.tile([128, 128], F32)
            masks.make_identity(nc, ident_f32[:])

            oh_sb = []
            for i, (r0, rn) in enumerate(TBLOCKS):
                t = constp.tile([rn, N], FP8, tag=f"oh{i}")
                nc.sync.dma_start(t[:], ohs.ap()[r0 : r0 + rn, :])
                oh_sb.append(t)

            fbd_sb = []
            for i, (r0, rn) in enumerate(KCHUNKS):
                t = constp.tile([rn, NK], F32, tag=f"fbd{i}")
                nc.sync.dma_start(t[:], fbd.ap()[r0 : r0 + rn, :])
                fbd_sb.append(t)

            bsh_sb = constp.tile([N, NSUB * NK], F32, tag="bsh")
            nc.sync.dma_start(bsh_sb[:], bsh.ap())

            hvg = grandp.tile([128, NSUB], F32, tag="hvg")
            fg = grandp.tile([128, NSUB * NK], F32, tag="fg")

            with (
                tc.tile_pool(name="tabs", bufs=3) as tabp,
                tc.tile_pool(name="wts", bufs=3) as wp,
                tc.tile_pool(name="binds", bufs=3) as bindp,
                tc.tile_pool(name="folds", bufs=2) as foldp,
                tc.tile_pool(name="trig", bufs=3) as trp,
                tc.tile_pool(name="psA", bufs=2, space="PSUM") as psa,
                tc.tile_pool(name="psB", bufs=2, space="PSUM") as psb,
            ):
                for c in range(CH):
                    cs = slice(c * CW, (c + 1) * CW)

                    # ---- DMA this chunk's tables and weights -------------
                    tt = []
                    for i, (r0, rn) in enumerate(TBLOCKS):
                        t = tabp.tile([rn, CW], FP8, tag=f"tab{i}")
                        nc.sync.dma_start(t[:], tabs.ap()[r0 : r0 + rn, cs])
                        tt.append(t)
                    wts = []
                    for i, (r0, rn) in enumerate(KCHUNKS):
                        t = wp.tile([rn, CW], F32, tag=f"w{i}")
                        nc.sync.dma_start(t[:], wstk.ap()[r0 : r0 + rn, cs])
                        wts.append(t)

                    # ---- phase A: lookups, bind, window product ----------
                    ps3 = psa.tile([128, CW], F32, tag="ps3")
                    nc.tensor.matmul(ps3[:], oh_sb[0][:], tt[0][:], start=True, stop=False)
                    nc.tensor.matmul(ps3[:], oh_sb[1][:], tt[1][:], start=False, stop=False)
                    nc.tensor.matmul(ps3[:], oh_sb[2][:], tt[2][:], start=False, stop=True)
                    ptl = psa.tile([128, CW], F32, tag="ptl")
                    nc.tensor.matmul(ptl[:], oh_sb[3][:], tt[3][:], start=True, stop=True)

                    tl_sb = bindp.tile([128, CW], BF, tag="tl_sb")
                    nc.scalar.copy(tl_sb[:], ptl[:])
                    bind_sb = bindp.tile([128, CW], BF, tag="bind_sb")
                    nc.vector.tensor_mul(bind_sb[:], ps3[:], tl_sb[:])

                    pbt = psa.tile([128, CW], BF, tag="pbt")
                    for s in range(4):
                        ss = slice(s * 128, (s + 1) * 128)
                        nc.tensor.matmul(
                            pbt[:, ss], bind_sb[:, ss], ident_bf[:], is_transpose=True
                        )
                    bt_sb = bindp.tile([128, CW], BF, tag="bt_sb")
                    nc.scalar.copy(bt_sb[:], pbt[:])

                    # pairwise fold over the window dim (free axis, 4 blocks)
                    src = bt_sb[:].rearrange("p (s n) -> p s n", s=4)
                    w = 64
                    while w >= 1:
                        if w == 1:
                            dst_ap = hvg[:, c * 4 : c * 4 + 4].rearrange(
                                "p (s n) -> p s n", n=1
                            )
                        else:
                            t_new = foldp.tile([128, 4 * w], F32, tag=f"fold{w}")
                            dst_ap = t_new[:].rearrange("p (s n) -> p s n", s=4)
                        nc.vector.tensor_mul(
                            dst_ap, src[:, :, 0:w], src[:, :, w : 2 * w]
                        )
                        if w > 1:
                            src = dst_ap
                        w //= 2

                    # ---- phase B: sinusoid einsum, W chunk stationary ----
                    ppt = psb.tile([128, 4 * NK], F32, tag="ppt")
                    for s in range(4):
                        ds = slice(s * 128, (s + 1) * 128)
                        for i in range(5):
                            nc.tensor.matmul(
                                ppt[:, s * NK : (s + 1) * NK],
                                wts[i][:, ds],
                                fbd_sb[i][:],
                                start=(i == 0),
                                stop=(i == 4),
                            )

                    fs = slice(c * 4 * NK, (c + 1) * 4 * NK)
                    # range reduction in cycle units: r = u - rint(u) in
                    # [-0.5, 0.5] (DVE f32->int32 copy rounds half-to-even,
                    # and the subtraction is exact), then Sin(2*pi*r) =
                    # sin(2*pi*u) on ScalarE's [-pi, pi] domain.
                    u = trp.tile([128, 4 * NK], F32, tag="u")
                    nc.vector.tensor_scalar_mul(u[:], ppt[:], float(_INV_2PI))
                    i1 = trp.tile([128, 4 * NK], I32, tag="i1")
                    nc.vector.tensor_copy(i1[:], u[:])
                    m1 = trp.tile([128, 4 * NK], F32, tag="m1")
                    nc.vector.tensor_sub(m1[:], u[:], i1[:])
                    nc.vector.tensor_scalar_mul(m1[:], m1[:], float(_TWO_PI))
                    s1 = trp.tile([128, 4 * NK], F32, tag="s1")
                    nc.scalar.activation(
                        s1[:], m1[:], mybir.ActivationFunctionType.Sin
                    )
                    u2 = trp.tile([128, 4 * NK], F32, tag="u2")
                    nc.vector.tensor_add(u2[:], u[:], bsh_sb[:, fs])
                    i2 = trp.tile([128, 4 * NK], I32, tag="i2")
                    nc.vector.tensor_copy(i2[:], u2[:])
                    m2 = trp.tile([128, 4 * NK], F32, tag="m2")
                    nc.vector.tensor_sub(m2[:], u2[:], i2[:])
                    nc.vector.tensor_scalar_mul(m2[:], m2[:], float(_TWO_PI))
                    s2 = trp.tile([128, 4 * NK], F32, tag="s2")
                    nc.scalar.activation(
                        s2[:], m2[:], mybir.ActivationFunctionType.Sin
                    )
                    nc.vector.tensor_mul(fg[:, fs], s2[:], s1[:])

            # ---------------- combine + hard quantize -------------------------
            with (
                tc.tile_pool(name="comb", bufs=1) as cp,
                tc.tile_pool(name="psC", bufs=1, space="PSUM") as psc,
            ):
                f3 = fg[:].rearrange("p (s k) -> p s k", k=NK)

                def f(k):
                    return f3[:, :, k : k + 1]

                def tmp(tag):
                    return cp.tile([128, NSUB], F32, tag=tag, name=tag)

                a1 = tmp("a1")
                a1v = a1[:].rearrange("p (s k) -> p s k", k=1)
                nc.vector.tensor_add(a1v, f(6), f(21))
                nc.vector.tensor_add(a1v, a1v, f(23))
                q1 = tmp("q1")
                q1v = q1[:].rearrange("p (s k) -> p s k", k=1)
                hvv = hvg[:].rearrange("p (s k) -> p s k", k=1)
                nc.vector.tensor_mul(q1v, hvv, a1v)
                a2 = tmp("a2")
                a2v = a2[:].rearrange("p (s k) -> p s k", k=1)
                nc.vector.tensor_add(a2v, f(9), f(10))
                nc.vector.tensor_mul(q1v, q1v, a2v)
                for k in (11, 12, 17, 18):
                    nc.vector.tensor_mul(q1v, q1v, f(k))

                a3 = tmp("a3")
                a3v = a3[:].rearrange("p (s k) -> p s k", k=1)
                nc.vector.tensor_add(a3v, f(6), f(10))
                nc.vector.tensor_add(a3v, a3v, f(11))
                nc.vector.tensor_add(a3v, a3v, f(12))
                p2 = tmp("p2")
                p2v = p2[:].rearrange("p (s k) -> p s k", k=1)
                nc.vector.tensor_mul(p2v, f(0), f(1))
                for k in (2, 3, 4, 5):
                    nc.vector.tensor_mul(p2v, p2v, f(k))
                q2 = tmp("q2")
                q2v = q2[:].rearrange("p (s k) -> p s k", k=1)
                nc.vector.tensor_mul(q2v, hvv, a3v)
                nc.vector.tensor_mul(q2v, q2v, p2v)

                comb = tmp("comb")
                nc.vector.tensor_add(comb[:], q1[:], q2[:])
                outq = tmp("outq")
                nc.vector.tensor_scalar(outq[:], comb[:], 0.0, 2.0, AF.is_gt, AF.mult)
                nc.vector.tensor_scalar(outq[:], outq[:], -1.0, None, AF.add)

                pso = psc.tile([NSUB, 128], F32, tag="pso")
                nc.tensor.matmul(
                    pso[:], outq[:], ident_f32[:], is_transpose=True
                )
                out_sb = cp.tile([NSUB, 128], F32, tag="out_sb")
                nc.scalar.copy(out_sb[:], pso[:])
                nc.sync.dma_start(out.ap(), out_sb[:])

    nc.compile()
    return nc


def _get_nc():
    if "nc" not in _nc_cache:
        _nc_cache["nc"] = _build_nc()
    return _nc_cache["nc"]


def _value_to_index(x, low, high, num):
    """Bit-identical (f32 elementwise IEEE ops) to the reference's jnp math."""
    x = x.astype(np.float32)
    xc = np.clip(x, np.float32(low), np.float32(high))
    t = (xc - np.float32(low)) / np.float32(high - low) * np.float32(num - 1)
    idx = np.round(t)  # round-half-even, same as jnp.round
    return np.clip(idx, 0, num - 1).astype(np.int32)


def prepare_in_maps(
    input,
    feat,
    level_x,
    level_y,
    level_z,
    level_t,
    W_big,
    b_big,
    W_small,
    b_small,
):
    ix = _value_to_index(input[:, 1], -5.0, 5.0, LEVELS)
    iy = _value_to_index(input[:, 2], -5.0, 5.0, LEVELS)
    iz = _value_to_index(input[:, 3], -5.0, 5.0, LEVELS)
    it = _value_to_index(input[:, 0], 0.0, float(TIMESTAMPS), TIMESTAMPS)

    # stacked one-hots [428, N] fp8 (0/1 exact)
    ohs = np.zeros((TROWS, N), dtype=FP8NP)
    for bi, idx in enumerate((ix, iy, iz, it)):
        r0 = TBLOCKS[bi][0]
        ohs[r0 + idx, np.arange(N)] = 1

    featb = feat[:546].reshape(6, 91).astype(np.float32)
    feats = feat[546:600].reshape(18, 3).astype(np.float32)
    fbd = np.zeros((KROWS, NK), dtype=np.float32)
    for k in range(6):
        fbd[k * 91 : (k + 1) * 91, k] = featb[k]
    for k in range(18):
        fbd[546 + k * 3 : 546 + (k + 1) * 3, 6 + k] = feats[k]

    def padD(a):
        w = [(0, 0)] * a.ndim
        w[-1] = (0, DP - D)
        return np.pad(a, w)

    # stacked tables [428, DP] fp8 (+-1 exact)
    tabs = padD(
        np.concatenate([level_x, level_y, level_z, level_t], axis=0)
    ).astype(FP8NP)

    # W stack [600, DP] f32: rows = (kernel-major, in-feature) of W_big/W_small
    wb = np.ascontiguousarray(W_big.transpose(0, 2, 1)).reshape(546, D)
    ws = np.ascontiguousarray(W_small.transpose(0, 2, 1)).reshape(54, D)
    wstk = padD(np.concatenate([wb, ws], axis=0)).astype(np.float32)

    # b shift in cycles (+0.25 for the cos->sin shift), d-on-partitions layout
    ball = np.concatenate([b_big, b_small], axis=0).astype(np.float64)
    bsh_full = padD((ball / (2.0 * np.pi) + 0.25).astype(np.float32))  # [24, DP]

    in_maps = []
    for ci in range(NCORES):
        ds = slice(ci * DC, (ci + 1) * DC)
        bs = (
            bsh_full[:, ds]
            .reshape(NK, NSUB, 128)
            .transpose(2, 1, 0)
            .reshape(128, NSUB * NK)
        )
        in_maps.append(
            {
                "tabs": np.ascontiguousarray(tabs[:, ds]),
                "ohs": ohs,
                "wstk": np.ascontiguousarray(wstk[:, ds]),
                "fbd": fbd,
                "bsh": np.ascontiguousarray(bs),
            }
        )
    return in_maps


def kernel(**inputs):
    nc = _get_nc()
    in_maps = prepare_in_maps(**inputs)
    _nc_cache["last_in_maps"] = in_maps
    res = bass_utils.run_bass_kernel_spmd(nc, in_maps, core_ids=list(range(NCORES)))
    shards = [res.results[ci]["out"].reshape(-1) for ci in range(NCORES)]
    return np.concatenate(shards)[:D].astype(np.float32)
